# revision 41
# baseline (speedup 1.0000x reference)
"""GCN (3-layer, PyG GCNConv-style) forward on 8 Trainium2 NeuronCores.

Strategy: data-parallel over the 64 graphs (8 graphs per core).  All
matmuls run fp8e4m3 with MatmulPerfMode.DoubleRow (two 128-row k-tiles
per instruction, 0.5 cycles/row = 4x the fp32r FLOP rate):
  - message passing is a dense normalized-adjacency matmul, A^T streamed
    fp8 (quantization of A costs ~3e-4 on the logits);
  - weight-side matmuls pair [W8 | (W-W8)8] as the two k-tiles with the
    activation broadcast via a stride-0 AP, recovering ~bf16 weight
    precision at fp8 speed (plain fp8 weights would cost 4e-2, weight
    error being systematic across nodes; compensated total is 7.7e-3
    vs the 2e-2 gate).

Host-side prep: the feature gather from the 500k-row table and the dense
A^T build happen on the host; the device receives per-graph feature
tiles (fp8, feature-major [128, 2048]) and A^T tiles (fp8, [128
src-part, 16 src-chunk, 2048 dst] swizzle), both on the SP DGE queue in
need order (features first; graph 0's A^T quartered so DoubleRow pairs
start before the full 4MB lands).  Per layer on device:
    h   = x @ W8 + x @ dW8   (16 DR matmuls, 4-chunk PSUM groups,
                              bulk-cast to fp8 on DVE)
    x'  = relu(A @ h + b)    (32 fp8 DR matmuls into 4 psum strips,
                              relu+bias on ACT writing fp8)
The layer orientations alternate (feat-major <-> node-major) so no
transposes are needed anywhere.  Emission is software-pipelined: strips
finish staggered, each strip's successor work is deferred one strip,
trailing pieces flush inside the next pair loop, and the next graph's
residual + layer-0 h production weave into the current graph's last
layer as PE filler.  The device emits only the per-graph node-sum
accumulators (ACT accum_out); the 64x2 logits head and the log_softmax
run on the host.
"""

import os
import sys

for _p in ("/opt/trn_rl_repo", "/root/.axon_site/_ro/trn_rl_repo"):
    if os.path.isdir(_p) and _p not in sys.path:
        sys.path.insert(0, _p)

import numpy as np
import ml_dtypes

import concourse.bass as bass
import concourse.bacc as bacc
import concourse.mybir as mybir
import concourse.tile as tile
from concourse import bass2jax

G, N, E = 64, 2048, 32768
D = H = 128
O = 2
ALL = 500_000
P = 128
N_CORES = 8
GPC = G // N_CORES          # graphs per core
NCH = N // P                # 128-row chunks per graph (16)

f32 = mybir.dt.float32
bf16 = mybir.dt.bfloat16
f8 = mybir.dt.float8e4

E4NP = ml_dtypes.float8_e4m3      # == mybir.dt.np(float8e4)
BFNP = ml_dtypes.bfloat16

DR = mybir.MatmulPerfMode.DoubleRow
RELU = mybir.ActivationFunctionType.Relu


def _build_program(n_layers: int):
    nc = bacc.Bacc("TRN2", target_bir_lowering=False, debug=False,
                   num_devices=N_CORES)

    # packed fp8 weight-compensation pairs [W8 | (W - W8)8]: wres, gw[l]s,
    # wfc.  Every W-side matmul runs fp8 DoubleRow with the input broadcast
    # (stride-0) across the two k-tiles, recovering ~bf16 weight precision
    # at 2x the bf16 matmul rate.
    NW = 2 + n_layers
    x0 = nc.dram_tensor("x0", [P, GPC * N], f8, kind="ExternalInput")
    at = nc.dram_tensor("at", [GPC * P, NCH * N], f8, kind="ExternalInput")
    wpk = nc.dram_tensor("wpk", [P, NW * 2 * H], f8, kind="ExternalInput")
    bpk = nc.dram_tensor("bpk", [P, NW], f32, kind="ExternalInput")
    macc_out = nc.dram_tensor("macc_out", [P, GPC * 4], f32,
                              kind="ExternalOutput")

    with tile.TileContext(nc) as tc:
        with tc.tile_pool(name="const", bufs=1) as const, \
             tc.tile_pool(name="apool", bufs=4) as apool, \
             tc.tile_pool(name="aqpool", bufs=1) as aqpool, \
             tc.tile_pool(name="inpool", bufs=4) as inpool, \
             tc.tile_pool(name="xpool", bufs=4) as xpool, \
             tc.tile_pool(name="x1pool", bufs=4) as x1pool, \
             tc.tile_pool(name="hpool", bufs=4) as hpool, \
             tc.tile_pool(name="fpool", bufs=2) as fpool, \
             tc.tile_pool(name="hps", bufs=2, space="PSUM") as hps, \
             tc.tile_pool(name="sps", bufs=2, space="PSUM") as sps, \
             tc.tile_pool(name="aps", bufs=1, space="PSUM") as aps:

            # ---- constants: two packed DMAs ----
            wpk_sb = const.tile([P, NW * 2, H], f8)
            nc.gpsimd.dma_start(
                out=wpk_sb[:],
                in_=wpk[:].rearrange("p (s n) -> p s n", s=NW * 2))
            bpk_sb = const.tile([P, NW], f32)
            nc.gpsimd.dma_start(out=bpk_sb[:], in_=bpk[:])
            wres_pr = wpk_sb[:, 0:2, :]
            wfc_pr = wpk_sb[:, (NW - 1) * 2:NW * 2, :]
            bres_sb = bpk_sb[:, 0:1]
            bfc_sb = bpk_sb[:, NW - 1:NW]
            macc = const.tile([P, GPC * 4], f32)
            zeros_sb = const.tile([P, 512], f32)
            nc.vector.memset(zeros_sb[:], 0.0)

            def bcast(ap2d, ncols):
                return ap2d.unsqueeze(1).broadcast_to((P, 2, ncols))

            def dma_at(g):
                """A^T DMA on the SP queue.  Graph 0's is quartered so its
                first DoubleRow pairs can start before the full 4MB lands."""
                if g == 0:
                    ats = []
                    for qq in range(4):
                        t = aqpool.tile([P, 4, N], f8, tag=f"atq{qq}",
                                        name=f"at0_{qq}")
                        nc.sync.dma_start(
                            out=t[:],
                            in_=at[0:P, qq * 4 * N:(qq + 1) * 4 * N].rearrange(
                                "p (s n) -> p s n", s=4))
                        ats.append(t)
                    return ats
                t = apool.tile([P, NCH, N], f8, tag="at", name=f"at{g}")
                nc.sync.dma_start(
                    out=t[:],
                    in_=at[g * P:(g + 1) * P, :].rearrange(
                        "p (s n) -> p s n", s=NCH))
                return [t]

            def at_pair(ats, j, q):
                if len(ats) == 4:
                    t, jj = ats[j // 2], (j % 2) * 2
                else:
                    t, jj = ats[0], 2 * j
                return t[:, jj:jj + 2, q * 512:(q + 1) * 512]

            def relu_bias(out, in_, bias, q, accum=None):
                """relu(x + b): ACT for even q, DVE for odd q, so each
                phase's strips drain on two engines in parallel.  The DVE
                accumulating path uses scalar_tensor_tensor, whose
                accum_out is a true sum."""
                if q % 2 == 0:
                    nc.scalar.activation(out=out, in_=in_, func=RELU,
                                         bias=bias, accum_out=accum)
                elif accum is None:
                    nc.vector.tensor_scalar(
                        out=out, in0=in_, scalar1=bias, scalar2=0.0,
                        op0=mybir.AluOpType.add, op1=mybir.AluOpType.max)
                else:
                    nc.vector.scalar_tensor_tensor(
                        out=out, in0=in_, scalar=bias, in1=zeros_sb[:],
                        op0=mybir.AluOpType.add, op1=mybir.AluOpType.max,
                        accum_out=accum)

            def emit_res_q(g, xT, x1T, q):
                """Residual strip q for graph g: fp8 DR matmul + relu."""
                rp = sps.tile([P, 512], f32, tag="sps", name=f"rp{g}_{q}")
                nc.tensor.matmul(out=rp[:], lhsT=wres_pr,
                                 rhs=bcast(xT[:, q * 512:(q + 1) * 512], 512),
                                 start=True, stop=True, perf_mode=DR)
                relu_bias(x1T[:, q * 512:(q + 1) * 512], rp[:], bres_sb,
                          q + 1)

            def emit_hgrp_q(g, l, x_src, h8t, q):
                """h-group q of layer l (chunks 4q..4q+3) + fp8 bulk cast."""
                hp = hps.tile([P, 512], f32, tag="hps", name=f"hp{g}_{l}_{q}")
                for c in range(4):
                    j = q * 4 + c
                    nc.tensor.matmul(
                        out=hp[:, c * H:(c + 1) * H],
                        lhsT=bcast(x_src[:, j * P:(j + 1) * P], P),
                        rhs=wpk_sb[:, (1 + l) * 2:(2 + l) * 2, :],
                        start=(c == 0), stop=(c == 3), perf_mode=DR)
                cast_out = h8t[:, q * 4:(q + 1) * 4, :].rearrange(
                    "p s f -> p (s f)")
                if q % 2 == 0:
                    nc.vector.tensor_copy(out=cast_out, in_=hp[:])
                else:
                    nc.scalar.activation(
                        out=cast_out, in_=hp[:],
                        func=mybir.ActivationFunctionType.Copy)

            def emit_fc_q(g, xn, x1T, q):
                """fc1 strip q: two bf16 matmuls accumulating (layer output
                + residual), then ACT relu + node-sum into macc."""
                fp = sps.tile([P, 512], f32, tag="sps", name=f"fp{g}_{q}")
                nc.tensor.matmul(out=fp[:], lhsT=wfc_pr,
                                 rhs=bcast(xn[:, q * 512:(q + 1) * 512], 512),
                                 start=True, stop=False, perf_mode=DR)
                nc.tensor.matmul(out=fp[:], lhsT=wfc_pr,
                                 rhs=bcast(x1T[:, q * 512:(q + 1) * 512], 512),
                                 start=False, stop=True, perf_mode=DR)
                fcq = fpool.tile([P, 512], f32, tag="fcq", name=f"fc{g}_{q}")
                relu_bias(fcq[:], fp[:], bfc_sb, q,
                          accum=macc[:, g * 4 + q:g * 4 + q + 1])

            def dma_x(g):
                t = inpool.tile([P, N], f8, tag="xin", name=f"x0_{g}")
                nc.gpsimd.dma_start(out=t[:], in_=x0[:, g * N:(g + 1) * N])
                return t

            # ---- prologue: first pair's inputs + their residual/layer-0 h
            xT = {0: dma_x(0)}
            ats = {0: dma_at(0)}
            x1T = {}
            h8 = {}
            if GPC > 1:
                xT[1] = dma_x(1)
                ats[1] = dma_at(1)
            for g in range(min(2, GPC)):
                x1T[g] = x1pool.tile([P, N], f8, tag="x1", name=f"x1_{g}")
                h8[g] = hpool.tile([P, NCH, H], f8, tag="h", name=f"h{g}_0")
                for q in range(4):
                    emit_res_q(g, xT[g], x1T[g], q)
                    emit_hgrp_q(g, 0, xT[g], h8[g], q)

            # Two-graph-deep software pipeline: the layers of a graph pair
            # alternate on PE, so each graph's relu -> h-group -> cast chain
            # drains while the partner graph's DoubleRow pairs run.
            # `pending[g]` (trailing h-group) flushes at pair j=1 of graph
            # g's NEXT layer (one full partner-layer later); fc strips flush
            # one per pair-j in whatever pair loop comes next.
            pending = {g: [] for g in range(GPC)}
            pending_fc = []

            def make_pre(g):
                x1T[g] = x1pool.tile([P, N], f8, tag="x1", name=f"x1_{g}")
                h8[g] = hpool.tile([P, NCH, H], f8, tag="h", name=f"h{g}_0")

                def pre_piece(q, _g=g):
                    emit_res_q(_g, xT[_g], x1T[_g], q)
                    emit_hgrp_q(_g, 0, xT[_g], h8[_g], q)
                return pre_piece

            npairs = (GPC + 1) // 2
            for k in range(npairs):
                pair = [g for g in (2 * k, 2 * k + 1) if g < GPC]
                nxt_pair = [g + 2 for g in pair if g + 2 < GPC]
                pre_pieces = {}
                for g in nxt_pair:
                    xT[g] = dma_x(g)
                for g in nxt_pair:
                    ats[g] = dma_at(g)
                    pre_pieces[g] = make_pre(g)

                if k == 0 and len(pair) == 2 and n_layers >= 2:
                    schedule = ([(pair[0], 0), (pair[0], 1), (pair[1], 0)]
                                + [(pair[0], l) for l in range(2, n_layers)]
                                + [(pair[1], l) for l in range(1, n_layers)])
                else:
                    schedule = [(g, l) for l in range(n_layers) for g in pair]
                for g, l in schedule:
                    last = (l == n_layers - 1)
                    if True:
                        ps_l = [aps.tile([P, 512], f32, tag=f"aps{q}",
                                         name=f"as{g}_{l}_{q}")
                                for q in range(4)]
                        for j in range(6):
                            hj = h8[g][:, 2 * j:2 * j + 2, :]
                            for q in range(4):
                                nc.tensor.matmul(
                                    out=ps_l[q][:], lhsT=hj,
                                    rhs=at_pair(ats[g], j, q),
                                    start=(j == 0), stop=False, perf_mode=DR)
                            if j == 1:
                                for fn in pending[g]:
                                    fn()
                                pending[g] = []
                            if 1 <= j <= 4 and pending_fc:
                                pending_fc[j - 1]()
                                if j == 4:
                                    pending_fc = []
                            if last and g + 2 in pre_pieces and 2 <= j <= 5:
                                pre_pieces[g + 2](j - 2)
                        xn = xpool.tile([P, N], f8, tag="x", name=f"x{g}_{l}")
                        if not last:
                            h8n = hpool.tile([P, NCH, H], f8, tag="h",
                                             name=f"h{g}_{l + 1}")

                        def deferred(q, _l=l, _xn=xn, _g=g,
                                     _h8n=(None if last else h8n)):
                            emit_hgrp_q(_g, _l + 1, _xn, _h8n, q)

                        very_last = (g == GPC - 1)
                        for q in range(4):
                            for j in (6, 7):
                                nc.tensor.matmul(
                                    out=ps_l[q][:],
                                    lhsT=h8[g][:, 2 * j:2 * j + 2, :],
                                    rhs=at_pair(ats[g], j, q),
                                    start=False, stop=(j == 7), perf_mode=DR)
                            relu_bias(xn[:, q * 512:(q + 1) * 512],
                                      ps_l[q][:], bpk_sb[:, 1 + l:2 + l], q)
                            if q >= 1 and not last:
                                deferred(q - 1)
                            if q >= 1 and last and very_last:
                                emit_fc_q(g, xn, x1T[g], q - 1)
                        if not last:
                            pending[g].append(lambda _d=deferred: _d(3))
                            h8[g] = h8n
                        elif very_last:
                            emit_fc_q(g, xn, x1T[g], 3)
                        else:
                            pending_fc = pending_fc + [
                                (lambda _q=q2, _xn=xn, _x1=x1T[g], _g=g:
                                 emit_fc_q(_g, _xn, _x1, _q))
                                for q2 in range(4)]

            for fns in pending.values():
                for fn in fns:
                    fn()
            for fn in pending_fc:
                fn()
            nc.sync.dma_start(out=macc_out[:], in_=macc[:])

    nc.compile()
    return nc


class _Runner:
    """Compile once, keep the jitted sharded executable for repeat calls."""

    def __init__(self, n_layers: int):
        import jax
        from jax.sharding import Mesh, PartitionSpec
        from jax.experimental.shard_map import shard_map

        self.jax = jax
        nc = _build_program(n_layers)
        self.nc = nc
        bass2jax.install_neuronx_cc_hook()

        in_names, out_names, out_avals, zero_outs = [], [], [], []
        pid_name = nc.partition_id_tensor.name if nc.partition_id_tensor else None
        for alloc in nc.m.functions[0].allocations:
            if not isinstance(alloc, mybir.MemoryLocationSet):
                continue
            name = alloc.memorylocations[0].name
            if alloc.kind == "ExternalInput":
                if name != pid_name:
                    in_names.append(name)
            elif alloc.kind == "ExternalOutput":
                out_names.append(name)
                shape = tuple(alloc.tensor_shape)
                dtype = mybir.dt.np(alloc.dtype)
                out_avals.append(jax.core.ShapedArray(shape, dtype))
                zero_outs.append(np.zeros(shape, dtype))
        self.in_names = list(in_names)
        self.out_names = out_names
        self.zero_outs = zero_outs
        n_params = len(in_names)
        all_names = in_names + out_names + ([pid_name] if pid_name else [])

        def _body(*args):
            operands = list(args)
            if pid_name is not None:
                operands.append(bass2jax.partition_id_tensor())
            return tuple(bass2jax._bass_exec_p.bind(
                *operands,
                out_avals=tuple(out_avals),
                in_names=tuple(all_names),
                out_names=tuple(out_names),
                lowering_input_output_aliases=(),
                sim_require_finite=True,
                sim_require_nnan=True,
                nc=nc,
            ))

        devices = jax.devices()[:N_CORES]
        mesh = Mesh(np.asarray(devices), ("core",))
        self.fn = jax.jit(
            shard_map(_body, mesh=mesh,
                      in_specs=(PartitionSpec("core"),) * (n_params + len(out_names)),
                      out_specs=(PartitionSpec("core"),) * len(out_names),
                      check_rep=False),
            keep_unused=True)

    def run(self, concat_inputs: list[np.ndarray]):
        jax = self.jax
        concat_zeros = [np.zeros((N_CORES * z.shape[0], *z.shape[1:]), z.dtype)
                        for z in self.zero_outs]
        outs = self.fn(*concat_inputs, *concat_zeros)
        jax.block_until_ready(outs)
        return {name: np.asarray(outs[i]) for i, name in enumerate(self.out_names)}


_RUNNERS: dict[int, _Runner] = {}


def _prepare_inputs(all_features, feature_index, edge_index,
                    lin_res_w, lin_res_b, gcn_w, gcn_b,
                    fc1_w, fc1_b, lin_w, lin_b, n_layers):
    """Build the concatenated (over cores, axis 0) device input list."""
    feats = np.asarray(all_features, np.float32)
    fi = np.asarray(feature_index).astype(np.int64)
    ei = np.asarray(edge_index).astype(np.int32)

    # host gather + transpose to feature-major fp8 [G, 128, 2048]
    x0_all = np.ascontiguousarray(
        feats[fi].transpose(0, 2, 1)).astype(E4NP)          # [G, D, N]

    # A^T per graph: accumulate duplicate (src,dst) cells, quantize fp8,
    # swizzle to [128 part, 16 chunk, 2048 dst].
    at_all = np.zeros((G, N * N), np.float32)
    diag_keys = (np.arange(N, dtype=np.int64) * (N + 1)).astype(np.int32)
    for g in range(G):
        src = ei[g, 0]
        dst = ei[g, 1]
        deg = np.bincount(dst, minlength=N).astype(np.float32) + 1.0
        dinv = 1.0 / np.sqrt(deg)
        coef = dinv[src] * dinv[dst]
        keys = np.concatenate([src.astype(np.int32) * N + dst, diag_keys])
        vals = np.concatenate([coef, dinv * dinv]).astype(np.float64)
        order = np.argsort(keys, kind="stable")
        ks, vs = keys[order], vals[order]
        first = np.empty(len(ks), bool)
        first[0] = True
        first[1:] = ks[1:] != ks[:-1]
        starts = np.nonzero(first)[0]
        sums = np.add.reduceat(vs, starts).astype(np.float32)
        np.put(at_all[g], ks[starts], sums)
    at8 = at_all.reshape(G, NCH, P, N).transpose(0, 2, 1, 3)  # [G,128,16,2048]
    at8 = np.ascontiguousarray(at8).astype(E4NP).reshape(G, P, NCH * N)

    # packed fp8 weight-compensation pairs [128, (2+L)*2*128]:
    # [W8 | (W-W8)8] blocks for wres | gw[0..L) | wfc
    NW = 2 + n_layers
    wpk = np.empty((P, NW * 2 * H), E4NP)

    def put_pair(b, W):
        Wf = np.asarray(W, np.float32)
        W8 = Wf.astype(E4NP)
        wpk[:, (2 * b) * H:(2 * b + 1) * H] = W8
        wpk[:, (2 * b + 1) * H:(2 * b + 2) * H] = (
            (Wf - W8.astype(np.float32)).astype(E4NP))

    put_pair(0, lin_res_w)
    for l in range(n_layers):
        put_pair(1 + l, gcn_w[l])
    put_pair(NW - 1, fc1_w)
    # packed biases [128, 2+L] f32: bres | gb[0..L) | bfc
    bpk = np.empty((P, NW), np.float32)
    bpk[:, 0] = np.asarray(lin_res_b, np.float32)
    for l in range(n_layers):
        bpk[:, 1 + l] = np.asarray(gcn_b[l], np.float32)
    bpk[:, NW - 1] = np.asarray(fc1_b, np.float32)

    per_core = {}
    per_core["x0"] = [np.ascontiguousarray(
        x0_all[c * GPC:(c + 1) * GPC].transpose(1, 0, 2)).reshape(P, GPC * N)
        for c in range(N_CORES)]
    per_core["at"] = [at8[c * GPC:(c + 1) * GPC].reshape(GPC * P, NCH * N)
                      for c in range(N_CORES)]
    per_core["wpk"] = [wpk] * N_CORES
    per_core["bpk"] = [bpk] * N_CORES
    return per_core


def kernel(all_features, feature_index, edge_index, action,
           lin_res_w, lin_res_b, gcn_w, gcn_b,
           fc1_w, fc1_b, lin_w, lin_b):
    n_layers = int(action) + 1
    assert 1 <= n_layers <= 3

    if n_layers not in _RUNNERS:
        _RUNNERS[n_layers] = _Runner(n_layers)
    runner = _RUNNERS[n_layers]

    per_core = _prepare_inputs(
        all_features, feature_index, edge_index,
        lin_res_w, lin_res_b, gcn_w, gcn_b, fc1_w, fc1_b, lin_w, lin_b,
        n_layers)

    concat = [np.concatenate(per_core[name], axis=0)
              for name in runner.in_names]
    outs = runner.run(concat)

    # host head: node-sums -> means -> logits -> log_softmax
    macc = outs["macc_out"].reshape(N_CORES, P, GPC, 4)
    means = macc.sum(axis=3).transpose(0, 2, 1).reshape(G, H) / N   # [G, H]
    lg = means @ np.asarray(lin_w, np.float32) + np.asarray(lin_b, np.float32)
    mx = lg.max(axis=1, keepdims=True)
    ls = lg - mx - np.log(np.exp(lg - mx).sum(axis=1, keepdims=True))
    return np.asarray(ls, np.float32), np.asarray(lg, np.float32)


# revision 42
# speedup vs baseline: 1.3308x; 1.3308x over previous
"""GCN (3-layer, PyG GCNConv-style) forward on 8 Trainium2 NeuronCores.

Strategy: data-parallel over the 64 graphs (8 graphs per core).  All
matmuls run fp8e4m3 with MatmulPerfMode.DoubleRow (two 128-row k-tiles
per instruction, 0.5 cycles/row = 4x the fp32r FLOP rate):
  - message passing is a dense normalized-adjacency matmul, A^T streamed
    fp8 (quantization of A costs ~3e-4 on the logits);
  - weight-side matmuls pair [W8 | (W-W8)8] as the two k-tiles with the
    activation broadcast via a stride-0 AP, recovering ~bf16 weight
    precision at fp8 speed (plain fp8 weights would cost 4e-2, weight
    error being systematic across nodes; compensated total is 7.7e-3
    vs the 2e-2 gate).

Host-side prep: the feature gather from the 500k-row table and the dense
A^T build happen on the host; the device receives per-graph feature
tiles (fp8, feature-major [128, 2048]) and A^T tiles (fp8, [128
src-part, 16 src-chunk, 2048 dst] swizzle), both on the SP DGE queue in
need order (features first; graph 0's A^T quartered so DoubleRow pairs
start before the full 4MB lands).  Per layer on device:
    h   = x @ W8 + x @ dW8   (16 DR matmuls, 4-chunk PSUM groups,
                              bulk-cast to fp8 on DVE)
    x'  = relu(A @ h + b)    (32 fp8 DR matmuls into 4 psum strips,
                              relu+bias on ACT writing fp8)
The layer orientations alternate (feat-major <-> node-major) so no
transposes are needed anywhere.  Emission is software-pipelined: strips
finish staggered, each strip's successor work is deferred one strip,
trailing pieces flush inside the next pair loop, and the next graph's
residual + layer-0 h production weave into the current graph's last
layer as PE filler.  The device emits only the per-graph node-sum
accumulators (ACT accum_out); the 64x2 logits head and the log_softmax
run on the host.
"""

import os
import sys

for _p in ("/opt/trn_rl_repo", "/root/.axon_site/_ro/trn_rl_repo"):
    if os.path.isdir(_p) and _p not in sys.path:
        sys.path.insert(0, _p)

import numpy as np
import ml_dtypes

import concourse.bass as bass
import concourse.bacc as bacc
import concourse.mybir as mybir
import concourse.tile as tile
from concourse import bass2jax

G, N, E = 64, 2048, 32768
D = H = 128
O = 2
ALL = 500_000
P = 128
N_CORES = 8
GPC = G // N_CORES          # graphs per core
NCH = N // P                # 128-row chunks per graph (16)

f32 = mybir.dt.float32
bf16 = mybir.dt.bfloat16
f8 = mybir.dt.float8e4

E4NP = ml_dtypes.float8_e4m3      # == mybir.dt.np(float8e4)
BFNP = ml_dtypes.bfloat16

DR = mybir.MatmulPerfMode.DoubleRow
RELU = mybir.ActivationFunctionType.Relu


def _build_program(n_layers: int):
    nc = bacc.Bacc("TRN2", target_bir_lowering=False, debug=False,
                   num_devices=N_CORES)

    # packed fp8 weight-compensation pairs [W8 | (W - W8)8]: wres, gw[l]s,
    # wfc.  Every W-side matmul runs fp8 DoubleRow with the input broadcast
    # (stride-0) across the two k-tiles, recovering ~bf16 weight precision
    # at 2x the bf16 matmul rate.
    NW = 2 + n_layers
    x0 = nc.dram_tensor("x0", [P, GPC * N], f8, kind="ExternalInput")
    at = nc.dram_tensor("at", [GPC * P, NCH * N], f8, kind="ExternalInput")
    wpk = nc.dram_tensor("wpk", [P, NW * 2 * H], f8, kind="ExternalInput")
    bpk = nc.dram_tensor("bpk", [P, NW], f32, kind="ExternalInput")
    macc_out = nc.dram_tensor("macc_out", [P, GPC * 4], f32,
                              kind="ExternalOutput")

    with tile.TileContext(nc) as tc:
        with tc.tile_pool(name="const", bufs=1) as const, \
             tc.tile_pool(name="apool", bufs=4) as apool, \
             tc.tile_pool(name="aqpool", bufs=1) as aqpool, \
             tc.tile_pool(name="inpool", bufs=4) as inpool, \
             tc.tile_pool(name="xpool", bufs=4) as xpool, \
             tc.tile_pool(name="x1pool", bufs=4) as x1pool, \
             tc.tile_pool(name="hpool", bufs=4) as hpool, \
             tc.tile_pool(name="fpool", bufs=2) as fpool, \
             tc.tile_pool(name="hps", bufs=2, space="PSUM") as hps, \
             tc.tile_pool(name="sps", bufs=2, space="PSUM") as sps, \
             tc.tile_pool(name="aps", bufs=1, space="PSUM") as aps:

            # ---- constants: two packed DMAs ----
            wpk_sb = const.tile([P, NW * 2, H], f8)
            nc.sync.dma_start(
                out=wpk_sb[:],
                in_=wpk[:].rearrange("p (s n) -> p s n", s=NW * 2))
            bpk_sb = const.tile([P, NW], f32)
            nc.sync.dma_start(out=bpk_sb[:], in_=bpk[:])
            wres_pr = wpk_sb[:, 0:2, :]
            wfc_pr = wpk_sb[:, (NW - 1) * 2:NW * 2, :]
            bres_sb = bpk_sb[:, 0:1]
            bfc_sb = bpk_sb[:, NW - 1:NW]
            macc = const.tile([P, GPC * 4], f32)
            zeros_sb = const.tile([P, 512], f32)
            nc.vector.memset(zeros_sb[:], 0.0)

            def bcast(ap2d, ncols):
                return ap2d.unsqueeze(1).broadcast_to((P, 2, ncols))

            def dma_at(g):
                """A^T DMA on the SP queue.  Graph 0's is quartered so its
                first DoubleRow pairs can start before the full 4MB lands."""
                if g == 0:
                    ats = []
                    for qq in range(4):
                        t = aqpool.tile([P, 4, N], f8, tag=f"atq{qq}",
                                        name=f"at0_{qq}")
                        nc.sync.dma_start(
                            out=t[:],
                            in_=at[0:P, qq * 4 * N:(qq + 1) * 4 * N].rearrange(
                                "p (s n) -> p s n", s=4))
                        ats.append(t)
                    return ats
                t = apool.tile([P, NCH, N], f8, tag="at", name=f"at{g}")
                nc.sync.dma_start(
                    out=t[:],
                    in_=at[g * P:(g + 1) * P, :].rearrange(
                        "p (s n) -> p s n", s=NCH))
                return [t]

            def at_pair(ats, j, q):
                if len(ats) == 4:
                    t, jj = ats[j // 2], (j % 2) * 2
                else:
                    t, jj = ats[0], 2 * j
                return t[:, jj:jj + 2, q * 512:(q + 1) * 512]

            def relu_bias(out, in_, bias, q, accum=None):
                """relu(x + b): ACT for even q, DVE for odd q, so each
                phase's strips drain on two engines in parallel.  The DVE
                accumulating path uses scalar_tensor_tensor, whose
                accum_out is a true sum."""
                if q % 2 == 0:
                    nc.scalar.activation(out=out, in_=in_, func=RELU,
                                         bias=bias, accum_out=accum)
                elif accum is None:
                    nc.vector.tensor_scalar(
                        out=out, in0=in_, scalar1=bias, scalar2=0.0,
                        op0=mybir.AluOpType.add, op1=mybir.AluOpType.max)
                else:
                    nc.vector.scalar_tensor_tensor(
                        out=out, in0=in_, scalar=bias, in1=zeros_sb[:],
                        op0=mybir.AluOpType.add, op1=mybir.AluOpType.max,
                        accum_out=accum)

            def emit_res_q(g, xT, x1T, q):
                """Residual strip q for graph g: fp8 DR matmul + relu."""
                rp = sps.tile([P, 512], f32, tag="sps", name=f"rp{g}_{q}")
                nc.tensor.matmul(out=rp[:], lhsT=wres_pr,
                                 rhs=bcast(xT[:, q * 512:(q + 1) * 512], 512),
                                 start=True, stop=True, perf_mode=DR)
                relu_bias(x1T[:, q * 512:(q + 1) * 512], rp[:], bres_sb,
                          q + 1)

            def emit_hgrp_q(g, l, x_src, h8t, q):
                """h-group q of layer l (chunks 4q..4q+3) + fp8 bulk cast."""
                hp = hps.tile([P, 512], f32, tag="hps", name=f"hp{g}_{l}_{q}")
                for c in range(4):
                    j = q * 4 + c
                    nc.tensor.matmul(
                        out=hp[:, c * H:(c + 1) * H],
                        lhsT=bcast(x_src[:, j * P:(j + 1) * P], P),
                        rhs=wpk_sb[:, (1 + l) * 2:(2 + l) * 2, :],
                        start=(c == 0), stop=(c == 3), perf_mode=DR)
                cast_out = h8t[:, q * 4:(q + 1) * 4, :].rearrange(
                    "p s f -> p (s f)")
                if q % 2 == 0:
                    nc.vector.tensor_copy(out=cast_out, in_=hp[:])
                else:
                    nc.scalar.activation(
                        out=cast_out, in_=hp[:],
                        func=mybir.ActivationFunctionType.Copy)

            def emit_fc_q(g, xn, x1T, q):
                """fc1 strip q: two bf16 matmuls accumulating (layer output
                + residual), then ACT relu + node-sum into macc."""
                fp = sps.tile([P, 512], f32, tag="sps", name=f"fp{g}_{q}")
                nc.tensor.matmul(out=fp[:], lhsT=wfc_pr,
                                 rhs=bcast(xn[:, q * 512:(q + 1) * 512], 512),
                                 start=True, stop=False, perf_mode=DR)
                nc.tensor.matmul(out=fp[:], lhsT=wfc_pr,
                                 rhs=bcast(x1T[:, q * 512:(q + 1) * 512], 512),
                                 start=False, stop=True, perf_mode=DR)
                fcq = fpool.tile([P, 512], f32, tag="fcq", name=f"fc{g}_{q}")
                relu_bias(fcq[:], fp[:], bfc_sb, q,
                          accum=macc[:, g * 4 + q:g * 4 + q + 1])

            def dma_x(g):
                t = inpool.tile([P, N], f8, tag="xin", name=f"x0_{g}")
                nc.sync.dma_start(out=t[:], in_=x0[:, g * N:(g + 1) * N])
                return t

            # ---- prologue: first pair's inputs + their residual/layer-0 h
            xT = {0: dma_x(0)}
            ats = {0: dma_at(0)}
            x1T = {}
            h8 = {}
            if GPC > 1:
                xT[1] = dma_x(1)
                ats[1] = dma_at(1)
            for g in range(min(2, GPC)):
                x1T[g] = x1pool.tile([P, N], f8, tag="x1", name=f"x1_{g}")
                h8[g] = hpool.tile([P, NCH, H], f8, tag="h", name=f"h{g}_0")
                for q in range(4):
                    emit_res_q(g, xT[g], x1T[g], q)
                    emit_hgrp_q(g, 0, xT[g], h8[g], q)

            # Two-graph-deep software pipeline: the layers of a graph pair
            # alternate on PE, so each graph's relu -> h-group -> cast chain
            # drains while the partner graph's DoubleRow pairs run.
            # `pending[g]` (trailing h-group) flushes at pair j=1 of graph
            # g's NEXT layer (one full partner-layer later); fc strips flush
            # one per pair-j in whatever pair loop comes next.
            pending = {g: [] for g in range(GPC)}
            pending_fc = []

            def make_pre(g):
                x1T[g] = x1pool.tile([P, N], f8, tag="x1", name=f"x1_{g}")
                h8[g] = hpool.tile([P, NCH, H], f8, tag="h", name=f"h{g}_0")

                def pre_piece(q, _g=g):
                    emit_res_q(_g, xT[_g], x1T[_g], q)
                    emit_hgrp_q(_g, 0, xT[_g], h8[_g], q)
                return pre_piece

            npairs = (GPC + 1) // 2
            for k in range(npairs):
                pair = [g for g in (2 * k, 2 * k + 1) if g < GPC]
                nxt_pair = [g + 2 for g in pair if g + 2 < GPC]
                pre_pieces = {}
                for g in nxt_pair:
                    xT[g] = dma_x(g)
                for g in nxt_pair:
                    ats[g] = dma_at(g)
                    pre_pieces[g] = make_pre(g)

                if k == 0 and len(pair) == 2 and n_layers >= 2:
                    schedule = ([(pair[0], 0), (pair[0], 1), (pair[1], 0)]
                                + [(pair[0], l) for l in range(2, n_layers)]
                                + [(pair[1], l) for l in range(1, n_layers)])
                else:
                    schedule = [(g, l) for l in range(n_layers) for g in pair]
                for g, l in schedule:
                    last = (l == n_layers - 1)
                    if True:
                        ps_l = [aps.tile([P, 512], f32, tag=f"aps{q}",
                                         name=f"as{g}_{l}_{q}")
                                for q in range(4)]
                        for j in range(6):
                            hj = h8[g][:, 2 * j:2 * j + 2, :]
                            for q in range(4):
                                nc.tensor.matmul(
                                    out=ps_l[q][:], lhsT=hj,
                                    rhs=at_pair(ats[g], j, q),
                                    start=(j == 0), stop=False, perf_mode=DR)
                            if j == 1:
                                for fn in pending[g]:
                                    fn()
                                pending[g] = []
                            if 1 <= j <= 4 and pending_fc:
                                pending_fc[j - 1]()
                                if j == 4:
                                    pending_fc = []
                            if last and g + 2 in pre_pieces and 2 <= j <= 5:
                                pre_pieces[g + 2](j - 2)
                        xn = xpool.tile([P, N], f8, tag="x", name=f"x{g}_{l}")
                        if not last:
                            h8n = hpool.tile([P, NCH, H], f8, tag="h",
                                             name=f"h{g}_{l + 1}")

                        def deferred(q, _l=l, _xn=xn, _g=g,
                                     _h8n=(None if last else h8n)):
                            emit_hgrp_q(_g, _l + 1, _xn, _h8n, q)

                        very_last = (g == GPC - 1)
                        for q in range(4):
                            for j in (6, 7):
                                nc.tensor.matmul(
                                    out=ps_l[q][:],
                                    lhsT=h8[g][:, 2 * j:2 * j + 2, :],
                                    rhs=at_pair(ats[g], j, q),
                                    start=False, stop=(j == 7), perf_mode=DR)
                            relu_bias(xn[:, q * 512:(q + 1) * 512],
                                      ps_l[q][:], bpk_sb[:, 1 + l:2 + l], q)
                            if q >= 1 and not last:
                                deferred(q - 1)
                            if q >= 1 and last and very_last:
                                emit_fc_q(g, xn, x1T[g], q - 1)
                        if not last:
                            pending[g].append(lambda _d=deferred: _d(3))
                            h8[g] = h8n
                        elif very_last:
                            emit_fc_q(g, xn, x1T[g], 3)
                        else:
                            pending_fc = pending_fc + [
                                (lambda _q=q2, _xn=xn, _x1=x1T[g], _g=g:
                                 emit_fc_q(_g, _xn, _x1, _q))
                                for q2 in range(4)]

            for fns in pending.values():
                for fn in fns:
                    fn()
            for fn in pending_fc:
                fn()
            nc.sync.dma_start(out=macc_out[:], in_=macc[:])

    nc.compile()
    return nc


class _Runner:
    """Compile once, keep the jitted sharded executable for repeat calls."""

    def __init__(self, n_layers: int):
        import jax
        from jax.sharding import Mesh, PartitionSpec
        from jax.experimental.shard_map import shard_map

        self.jax = jax
        nc = _build_program(n_layers)
        self.nc = nc
        bass2jax.install_neuronx_cc_hook()

        in_names, out_names, out_avals, zero_outs = [], [], [], []
        pid_name = nc.partition_id_tensor.name if nc.partition_id_tensor else None
        for alloc in nc.m.functions[0].allocations:
            if not isinstance(alloc, mybir.MemoryLocationSet):
                continue
            name = alloc.memorylocations[0].name
            if alloc.kind == "ExternalInput":
                if name != pid_name:
                    in_names.append(name)
            elif alloc.kind == "ExternalOutput":
                out_names.append(name)
                shape = tuple(alloc.tensor_shape)
                dtype = mybir.dt.np(alloc.dtype)
                out_avals.append(jax.core.ShapedArray(shape, dtype))
                zero_outs.append(np.zeros(shape, dtype))
        self.in_names = list(in_names)
        self.out_names = out_names
        self.zero_outs = zero_outs
        n_params = len(in_names)
        all_names = in_names + out_names + ([pid_name] if pid_name else [])

        def _body(*args):
            operands = list(args)
            if pid_name is not None:
                operands.append(bass2jax.partition_id_tensor())
            return tuple(bass2jax._bass_exec_p.bind(
                *operands,
                out_avals=tuple(out_avals),
                in_names=tuple(all_names),
                out_names=tuple(out_names),
                lowering_input_output_aliases=(),
                sim_require_finite=True,
                sim_require_nnan=True,
                nc=nc,
            ))

        devices = jax.devices()[:N_CORES]
        mesh = Mesh(np.asarray(devices), ("core",))
        self.fn = jax.jit(
            shard_map(_body, mesh=mesh,
                      in_specs=(PartitionSpec("core"),) * (n_params + len(out_names)),
                      out_specs=(PartitionSpec("core"),) * len(out_names),
                      check_rep=False),
            keep_unused=True)

    def run(self, concat_inputs: list[np.ndarray]):
        jax = self.jax
        concat_zeros = [np.zeros((N_CORES * z.shape[0], *z.shape[1:]), z.dtype)
                        for z in self.zero_outs]
        outs = self.fn(*concat_inputs, *concat_zeros)
        jax.block_until_ready(outs)
        return {name: np.asarray(outs[i]) for i, name in enumerate(self.out_names)}


_RUNNERS: dict[int, _Runner] = {}


def _prepare_inputs(all_features, feature_index, edge_index,
                    lin_res_w, lin_res_b, gcn_w, gcn_b,
                    fc1_w, fc1_b, lin_w, lin_b, n_layers):
    """Build the concatenated (over cores, axis 0) device input list."""
    feats = np.asarray(all_features, np.float32)
    fi = np.asarray(feature_index).astype(np.int64)
    ei = np.asarray(edge_index).astype(np.int32)

    # host gather + transpose to feature-major fp8 [G, 128, 2048]
    x0_all = np.ascontiguousarray(
        feats[fi].transpose(0, 2, 1)).astype(E4NP)          # [G, D, N]

    # A^T per graph: accumulate duplicate (src,dst) cells, quantize fp8,
    # swizzle to [128 part, 16 chunk, 2048 dst].
    at_all = np.zeros((G, N * N), np.float32)
    diag_keys = (np.arange(N, dtype=np.int64) * (N + 1)).astype(np.int32)
    for g in range(G):
        src = ei[g, 0]
        dst = ei[g, 1]
        deg = np.bincount(dst, minlength=N).astype(np.float32) + 1.0
        dinv = 1.0 / np.sqrt(deg)
        coef = dinv[src] * dinv[dst]
        keys = np.concatenate([src.astype(np.int32) * N + dst, diag_keys])
        vals = np.concatenate([coef, dinv * dinv]).astype(np.float64)
        order = np.argsort(keys, kind="stable")
        ks, vs = keys[order], vals[order]
        first = np.empty(len(ks), bool)
        first[0] = True
        first[1:] = ks[1:] != ks[:-1]
        starts = np.nonzero(first)[0]
        sums = np.add.reduceat(vs, starts).astype(np.float32)
        np.put(at_all[g], ks[starts], sums)
    at8 = at_all.reshape(G, NCH, P, N).transpose(0, 2, 1, 3)  # [G,128,16,2048]
    at8 = np.ascontiguousarray(at8).astype(E4NP).reshape(G, P, NCH * N)

    # packed fp8 weight-compensation pairs [128, (2+L)*2*128]:
    # [W8 | (W-W8)8] blocks for wres | gw[0..L) | wfc
    NW = 2 + n_layers
    wpk = np.empty((P, NW * 2 * H), E4NP)

    def put_pair(b, W):
        Wf = np.asarray(W, np.float32)
        W8 = Wf.astype(E4NP)
        wpk[:, (2 * b) * H:(2 * b + 1) * H] = W8
        wpk[:, (2 * b + 1) * H:(2 * b + 2) * H] = (
            (Wf - W8.astype(np.float32)).astype(E4NP))

    put_pair(0, lin_res_w)
    for l in range(n_layers):
        put_pair(1 + l, gcn_w[l])
    put_pair(NW - 1, fc1_w)
    # packed biases [128, 2+L] f32: bres | gb[0..L) | bfc
    bpk = np.empty((P, NW), np.float32)
    bpk[:, 0] = np.asarray(lin_res_b, np.float32)
    for l in range(n_layers):
        bpk[:, 1 + l] = np.asarray(gcn_b[l], np.float32)
    bpk[:, NW - 1] = np.asarray(fc1_b, np.float32)

    per_core = {}
    per_core["x0"] = [np.ascontiguousarray(
        x0_all[c * GPC:(c + 1) * GPC].transpose(1, 0, 2)).reshape(P, GPC * N)
        for c in range(N_CORES)]
    per_core["at"] = [at8[c * GPC:(c + 1) * GPC].reshape(GPC * P, NCH * N)
                      for c in range(N_CORES)]
    per_core["wpk"] = [wpk] * N_CORES
    per_core["bpk"] = [bpk] * N_CORES
    return per_core


def kernel(all_features, feature_index, edge_index, action,
           lin_res_w, lin_res_b, gcn_w, gcn_b,
           fc1_w, fc1_b, lin_w, lin_b):
    n_layers = int(action) + 1
    assert 1 <= n_layers <= 3

    if n_layers not in _RUNNERS:
        _RUNNERS[n_layers] = _Runner(n_layers)
    runner = _RUNNERS[n_layers]

    per_core = _prepare_inputs(
        all_features, feature_index, edge_index,
        lin_res_w, lin_res_b, gcn_w, gcn_b, fc1_w, fc1_b, lin_w, lin_b,
        n_layers)

    concat = [np.concatenate(per_core[name], axis=0)
              for name in runner.in_names]
    outs = runner.run(concat)

    # host head: node-sums -> means -> logits -> log_softmax
    macc = outs["macc_out"].reshape(N_CORES, P, GPC, 4)
    means = macc.sum(axis=3).transpose(0, 2, 1).reshape(G, H) / N   # [G, H]
    lg = means @ np.asarray(lin_w, np.float32) + np.asarray(lin_b, np.float32)
    mx = lg.max(axis=1, keepdims=True)
    ls = lg - mx - np.log(np.exp(lg - mx).sum(axis=1, keepdims=True))
    return np.asarray(ls, np.float32), np.asarray(lg, np.float32)
